# revision 8
# baseline (speedup 1.0000x reference)
"""Trainium2 Bass kernel for nn_DIE: per-pixel channel SE gate.

    h    = relu(W1 @ x[:, :, i, j])      # [B, 32, H, W]
    gate = sigmoid(W2 @ h)               # [B, 512, H, W]
    y    = gate * x

Sharding: pure data parallel over the batch dim (B=8 -> 8 cores).
Each core streams its [512, 192*192] slab through SBUF in DMA tiles.
All HBM I/O is bf16 (x quantized host-side, y dequantized host-side):
halves the HBM traffic vs fp32 for ~0.7% relative error, far inside
the 2e-2 gate.

Both matmuls only use 1/4 of the 128x128 PE array (mm1 writes 32
output channels; mm2 contracts over 32), so the PE is addressed as
four independent 32-strips via tile_position:
  - mm1 (column tiling): pixel chunk c of 512 goes to PE column tile
    (0, 32c); h for chunk c lands on PSUM partitions 32c..32c+31.
    The four chunks' matmuls stream concurrently.
  - mm2 (row tiling): chunk c's h is read back from SBUF partitions
    32c..32c+31 by PE row tile (32c, 0) with W2 replicated across the
    four partition strips; concurrent matmuls rebuild [128, 1024]
    gate-logit slabs whose PSUM writes stay bank-aligned (one full
    2KB bank per matmul -- matmul outputs must not cross banks).
  - an explicit nc.tensor.drain() guards each row->column tiling mode
    change (mode switches with in-flight matmuls wedge the array).
This cuts tensor-engine busy time several-fold -- which is what paces
the kernel, since the activity governor throttles the PE to ~50%
utilization most of the run. The slab layout lets the sigmoid run as
[128, 1024] activations (144 calls instead of 288; the scalar engine
pays a ~352-cycle pipeline bubble per instruction), and the relu and
the final gate*x multiply run on the DVE (2x bf16 mode). Deep rings
everywhere (gate PSUM x3, DMA tiles x4) keep all engines streaming
through throttle bursts.
"""

import sys

for _p in ("/opt/trn_rl_repo",):
    if _p not in sys.path:
        sys.path.insert(0, _p)

import ml_dtypes
import numpy as np

import concourse.bacc as bacc
import concourse.bass as bass
import concourse.mybir as mybir
from concourse import tile
from concourse.bass_utils import run_bass_kernel_spmd

B, C, H, W = 8, 512, 192, 192
R = 32            # C // RED
NPIX = H * W      # 36864 pixels per batch element
N_CORES = 8
PART = 128
G = C // PART     # 4 channel groups
CH = 512          # pixels per PE-tile chunk = one PSUM bank of fp32
DMA_N = 4 * CH    # pixels per DMA tile (2048)

F32 = mybir.dt.float32
BF16 = mybir.dt.bfloat16
AF = mybir.ActivationFunctionType
BF16_NP = ml_dtypes.bfloat16


def build(npix: int = NPIX, dma_n: int = DMA_N):
    """Build the per-core Bass program (SPMD: identical on all cores)."""
    assert npix % dma_n == 0
    # Small tiles at the head (prime the out-DMA stream sooner: the out
    # ring is the critical path and can only start after the first tile's
    # compute) and at the tail (let the final transfers drain sooner).
    half = dma_n // 2
    assert half % (2 * CH) == 0
    tile_sizes = [half] * 2 + [dma_n] * (npix // dma_n - 2) + [half] * 2
    assert sum(tile_sizes) == npix

    nc = bacc.Bacc("TRN2", target_bir_lowering=False, debug=False, num_devices=N_CORES)

    x_d = nc.dram_tensor("x", [C, npix], BF16, kind="ExternalInput").ap()
    w1t_d = nc.dram_tensor("w1t", [C, R], BF16, kind="ExternalInput").ap()  # W1.T
    # W2.T replicated on each 32-partition strip: w2r[32c+r, 128g+m] = W2[128g+m, r]
    w2r_d = nc.dram_tensor("w2r", [PART, C], BF16, kind="ExternalInput").ap()
    y_d = nc.dram_tensor("y", [C, npix], BF16, kind="ExternalOutput").ap()

    with tile.TileContext(nc) as tc:
        with (
            tc.tile_pool(name="wpool", bufs=1) as wpool,
            tc.tile_pool(name="xp", bufs=4) as xp,
            tc.tile_pool(name="hp", bufs=3) as hp,
            tc.tile_pool(name="gp", bufs=4) as gp,
            tc.tile_pool(name="op", bufs=4) as op_,
            tc.tile_pool(name="pp", bufs=2, space=bass.MemorySpace.PSUM) as pp,
        ):
            # Weights, loaded once.
            # w1t[p, g, r] = W1T[g*128+p, r]; w2r[32c+r, g, m] = W2T[r, g*128+m].
            w1t = wpool.tile([PART, G, R], BF16)
            nc.sync.dma_start(w1t[:], w1t_d.rearrange("(g p) r -> p g r", p=PART))
            w2r = wpool.tile([PART, G, PART], BF16)
            nc.sync.dma_start(w2r[:], w2r_d.rearrange("p (g m) -> p g m", m=PART))

            n0 = 0
            for ti, tn in enumerate(tile_sizes):
                nch = tn // CH
                xt = xp.tile([PART, G, tn], BF16, tag="xt")
                nc.sync.dma_start(
                    xt[:],
                    x_d[:, n0 : n0 + tn].rearrange("(g p) n -> p g n", p=PART),
                )

                # mm1, column-tiled: h for pixel chunk c sits on PSUM/SBUF
                # partitions 32c..32c+31; the nch chunks stream concurrently.
                # The PE tiling mode changes here (row -> column): drain the
                # in-flight row-mode matmuls first, or the array wedges.
                if ti:
                    nc.tensor.drain()
                hps = pp.tile([PART, CH], F32, tag="hps")
                for c in range(nch):
                    pc = 32 * c
                    sl = slice(c * CH, (c + 1) * CH)
                    for g in range(G):
                        nc.tensor.matmul(
                            hps[pc : pc + 32, :], w1t[:, g, :], xt[:, g, sl],
                            start=(g == 0), stop=(g == G - 1),
                            tile_position=(0, pc),
                        )
                hs = hp.tile([PART, CH], BF16, tag="hs")
                nc.vector.tensor_scalar_max(hs[:], hps[:], 0.0)

                ot = op_.tile([PART, G, tn], BF16, tag="ot")
                for g in range(G):
                    for hf in range(nch // 2):
                        # mm2, row-tiled: two concurrent 32-row matmuls fill a
                        # [128, 1024] gate-logit slab, one 2KB bank each.
                        gps = pp.tile([PART, 2 * CH], F32, tag="gps", bufs=3)
                        for i in range(2):
                            pc = 32 * (2 * hf + i)
                            nc.tensor.matmul(
                                gps[:, i * CH : (i + 1) * CH],
                                w2r[pc : pc + 32, g, :],
                                hs[pc : pc + 32, :],
                                start=True, stop=True,
                                tile_position=(pc, 0),
                            )
                        sl2 = slice(hf * 2 * CH, (hf + 1) * 2 * CH)
                        gs = gp.tile([PART, 2 * CH], BF16, tag="gs")
                        nc.scalar.activation(gs[:], gps[:], AF.Sigmoid)
                        nc.vector.tensor_mul(ot[:, g, sl2], gs[:], xt[:, g, sl2])

                # Drain assist: the sync ring's loads finish before the tail
                # stores, so route the last few stores onto it -- both HWDGE
                # rings then drain the output in parallel.
                out_eng = nc.sync if ti >= len(tile_sizes) - 3 else nc.scalar
                out_eng.dma_start(
                    y_d[:, n0 : n0 + tn].rearrange("(g p) n -> p g n", p=PART),
                    ot[:],
                )
                n0 += tn

    nc.compile()
    return nc


def _plausible(y: np.ndarray, x: np.ndarray) -> bool:
    """Cheap integrity check: y = sigmoid(.)*x implies |y| <= |x| (modulo
    bf16 rounding), finite everywhere, and y is never 0 where x isn't
    tiny (the gate can't underflow for this weight scale). Transient DMA
    corruption / stale pages violate these with near-certainty."""
    y = np.asarray(y, dtype=np.float32)
    x = np.asarray(x, dtype=np.float32)
    if not np.isfinite(y).all():
        return False
    ax = np.abs(x)
    if (np.abs(y) > ax * 1.01 + 1e-30).any():
        return False
    if np.count_nonzero((y == 0.0) & (ax > 1e-3)) > y.size // 1_000_000:
        return False
    return True


def kernel(x: np.ndarray, W1: np.ndarray, W2: np.ndarray, **run_kwargs):
    """Full-input entry point: shards batch over 8 cores, returns full output."""
    x = np.asarray(x)
    assert x.shape == (B, C, H, W), x.shape
    nc = build()

    w1t = np.ascontiguousarray(np.asarray(W1).T).astype(BF16_NP)  # [512, 32]
    w2r = np.ascontiguousarray(np.tile(np.asarray(W2).T, (4, 1))).astype(BF16_NP)
    x_bf = [
        np.ascontiguousarray(x[i].reshape(C, NPIX)).astype(BF16_NP)
        for i in range(N_CORES)
    ]
    in_maps = [{"x": x_bf[i], "w1t": w1t, "w2r": w2r} for i in range(N_CORES)]
    retries = 2 if not run_kwargs.get("trace") else 0
    for attempt in range(retries + 1):
        res = run_bass_kernel_spmd(nc, in_maps, list(range(N_CORES)), **run_kwargs)
        if all(
            _plausible(res.results[i]["y"], x_bf[i]) for i in range(N_CORES)
        ):
            break
    y = np.stack(
        [
            res.results[i]["y"].astype(np.float32).reshape(C, H, W)
            for i in range(N_CORES)
        ]
    )
    if run_kwargs:
        return y, res
    return y


# revision 10
# speedup vs baseline: 1.0805x; 1.0805x over previous
"""Trainium2 Bass kernel for nn_DIE: per-pixel channel SE gate.

    h    = relu(W1 @ x[:, :, i, j])      # [B, 32, H, W]
    gate = sigmoid(W2 @ h)               # [B, 512, H, W]
    y    = gate * x

Sharding: pure data parallel over the batch dim (B=8 -> 8 cores).
Each core streams its [512, 192*192] slab through SBUF in DMA tiles
of 2048 pixels (4KB contiguous runs per channel in bf16). All HBM I/O
is bf16 (x quantized host-side, y dequantized host-side): halves the
HBM traffic vs fp32 for ~0.7% relative error, far inside the 2e-2
gate.

Engine balance (the kernel is HBM-bound; compute engines are also
activity-throttled to ~50-70% utilization, so their busy time matters):
  - matmuls run untiled in bf16 with fp32 PSUM accumulation; every
    PSUM matmul output is exactly one 2KB bank (outputs must never
    cross a bank boundary).
  - gate logits accumulate into [128, 1024] two-bank PSUM slabs so the
    sigmoid runs as 144 big activations instead of 288 small ones (the
    scalar engine pays a ~352-cycle pipeline bubble per instruction).
  - relu and the final gate*x multiply run on the DVE (2x bf16 mode),
    keeping the scalar engine free for the sigmoid.
  - deep rings (gate PSUM x3, DMA tiles x4) keep every engine
    streaming through throttle bursts; shallow rings here cost ~100us.
"""

import sys

for _p in ("/opt/trn_rl_repo",):
    if _p not in sys.path:
        sys.path.insert(0, _p)

import ml_dtypes
import numpy as np

import concourse.bacc as bacc
import concourse.bass as bass
import concourse.mybir as mybir
from concourse import tile
from concourse.bass_utils import run_bass_kernel_spmd

B, C, H, W = 8, 512, 192, 192
R = 32            # C // RED
NPIX = H * W      # 36864 pixels per batch element
N_CORES = 8
PART = 128
G = C // PART     # 4 channel groups
CH = 512          # pixels per compute chunk = one PSUM bank of fp32
DMA_N = 4 * CH    # pixels per DMA tile (2048)

F32 = mybir.dt.float32
BF16 = mybir.dt.bfloat16
AF = mybir.ActivationFunctionType
BF16_NP = ml_dtypes.bfloat16


def build(npix: int = NPIX, dma_n: int = DMA_N):
    """Build the per-core Bass program (SPMD: identical on all cores)."""
    assert npix % dma_n == 0
    # Small tiles at the head (prime the out-DMA stream sooner: the out
    # ring is the critical path and can only start after the first tile's
    # compute) and at the tail (let the final transfers drain sooner).
    half = dma_n // 2
    assert half % (2 * CH) == 0
    tile_sizes = [half] * 2 + [dma_n] * (npix // dma_n - 2) + [half] * 2
    assert sum(tile_sizes) == npix

    nc = bacc.Bacc("TRN2", target_bir_lowering=False, debug=False, num_devices=N_CORES)

    x_d = nc.dram_tensor("x", [C, npix], BF16, kind="ExternalInput").ap()
    w1t_d = nc.dram_tensor("w1t", [C, R], BF16, kind="ExternalInput").ap()  # W1.T
    w2r_d = nc.dram_tensor("w2r", [PART, C], BF16, kind="ExternalInput").ap()
    y_d = nc.dram_tensor("y", [C, npix], BF16, kind="ExternalOutput").ap()

    with tile.TileContext(nc) as tc:
        with (
            tc.tile_pool(name="wpool", bufs=1) as wpool,
            tc.tile_pool(name="xp", bufs=4) as xp,
            tc.tile_pool(name="hp", bufs=8) as hp,
            tc.tile_pool(name="gp", bufs=6) as gp,
            tc.tile_pool(name="op", bufs=4) as op_,
            tc.tile_pool(name="pp", bufs=2, space=bass.MemorySpace.PSUM) as pp,
        ):
            # Weights, loaded once.
            # w1t[p, g, r] = W1T[g*128+p, r]; w2r[32c+r, g, m] = W2T[r, g*128+m]
            # (only strip 0 of the replicated W2 is used here).
            w1t = wpool.tile([PART, G, R], BF16)
            nc.sync.dma_start(w1t[:], w1t_d.rearrange("(g p) r -> p g r", p=PART))
            w2r = wpool.tile([PART, G, PART], BF16)
            nc.sync.dma_start(w2r[:], w2r_d.rearrange("p (g m) -> p g m", m=PART))

            n0 = 0
            for ti, tn in enumerate(tile_sizes):
                nch = tn // CH
                xt = xp.tile([PART, G, tn], BF16, tag="xt")
                nc.sync.dma_start(
                    xt[:],
                    x_d[:, n0 : n0 + tn].rearrange("(g p) n -> p g n", p=PART),
                )

                # mm1 per 512-pixel chunk; relu on the DVE.
                hss = []
                for k in range(nch):
                    sl = slice(k * CH, (k + 1) * CH)
                    hps = pp.tile([R, CH], F32, tag="hps")
                    for g in range(G):
                        nc.tensor.matmul(
                            hps[:], w1t[:, g, :], xt[:, g, sl],
                            start=(g == 0), stop=(g == G - 1),
                        )
                    hs = hp.tile([R, CH], BF16, tag="hs")
                    nc.vector.tensor_scalar_max(hs[:], hps[:], 0.0)
                    hss.append(hs)

                ot = op_.tile([PART, G, tn], BF16, tag="ot")
                for g in range(G):
                    for hf in range(nch // 2):
                        # Two chunk matmuls (same stationary W2 block) fill a
                        # [128, 1024] gate-logit slab, one 2KB bank each.
                        gps = pp.tile([PART, 2 * CH], F32, tag="gps", bufs=3)
                        for i in range(2):
                            nc.tensor.matmul(
                                gps[:, i * CH : (i + 1) * CH],
                                w2r[0:R, g, :],
                                hss[2 * hf + i][:],
                                start=True, stop=True,
                            )
                        sl2 = slice(hf * 2 * CH, (hf + 1) * 2 * CH)
                        gs = gp.tile([PART, 2 * CH], BF16, tag="gs")
                        nc.scalar.activation(gs[:], gps[:], AF.Sigmoid)
                        nc.vector.tensor_mul(ot[:, g, sl2], gs[:], xt[:, g, sl2])

                # Drain assist: the sync ring's loads finish before the tail
                # stores, so route the last few stores onto it -- both HWDGE
                # rings then drain the output in parallel.
                out_eng = nc.sync if ti >= len(tile_sizes) - 3 else nc.scalar
                out_eng.dma_start(
                    y_d[:, n0 : n0 + tn].rearrange("(g p) n -> p g n", p=PART),
                    ot[:],
                )
                n0 += tn

    nc.compile()
    return nc


def _plausible(y: np.ndarray, x: np.ndarray) -> bool:
    """Cheap integrity check: y = sigmoid(.)*x implies |y| <= |x| (modulo
    bf16 rounding), finite everywhere, and y is never 0 where x isn't
    tiny (the gate can't underflow for this weight scale). Transient DMA
    corruption / stale pages violate these with near-certainty."""
    y = np.asarray(y, dtype=np.float32)
    x = np.asarray(x, dtype=np.float32)
    if not np.isfinite(y).all():
        return False
    ax = np.abs(x)
    if (np.abs(y) > ax * 1.01 + 1e-30).any():
        return False
    if np.count_nonzero((y == 0.0) & (ax > 1e-3)) > y.size // 1_000_000:
        return False
    return True


def kernel(x: np.ndarray, W1: np.ndarray, W2: np.ndarray, **run_kwargs):
    """Full-input entry point: shards batch over 8 cores, returns full output."""
    x = np.asarray(x)
    assert x.shape == (B, C, H, W), x.shape
    nc = build()

    w1t = np.ascontiguousarray(np.asarray(W1).T).astype(BF16_NP)  # [512, 32]
    w2r = np.ascontiguousarray(np.tile(np.asarray(W2).T, (4, 1))).astype(BF16_NP)
    x_bf = [
        np.ascontiguousarray(x[i].reshape(C, NPIX)).astype(BF16_NP)
        for i in range(N_CORES)
    ]
    in_maps = [{"x": x_bf[i], "w1t": w1t, "w2r": w2r} for i in range(N_CORES)]
    retries = 2 if not run_kwargs.get("trace") else 0
    for attempt in range(retries + 1):
        res = run_bass_kernel_spmd(nc, in_maps, list(range(N_CORES)), **run_kwargs)
        if all(
            _plausible(res.results[i]["y"], x_bf[i]) for i in range(N_CORES)
        ):
            break
    y = np.stack(
        [
            res.results[i]["y"].astype(np.float32).reshape(C, H, W)
            for i in range(N_CORES)
        ]
    )
    if run_kwargs:
        return y, res
    return y


# revision 11
# speedup vs baseline: 1.0829x; 1.0022x over previous
"""Trainium2 Bass kernel for nn_DIE: per-pixel channel SE gate.

    h    = relu(W1 @ x[:, :, i, j])      # [B, 32, H, W]
    gate = sigmoid(W2 @ h)               # [B, 512, H, W]
    y    = gate * x

Sharding: pure data parallel over the batch dim (B=8 -> 8 cores).
Each core streams its [512, 192*192] slab through SBUF in DMA tiles
of 2048 pixels (4KB contiguous runs per channel in bf16). All HBM I/O
is bf16 (x quantized host-side, y dequantized host-side): halves the
HBM traffic vs fp32 for ~0.7% relative error, far inside the 2e-2
gate.

Engine balance (the kernel is HBM-bound; compute engines are also
activity-throttled to ~50-70% utilization, so their busy time matters):
  - matmuls run untiled in bf16 with fp32 PSUM accumulation; every
    PSUM matmul output is exactly one 2KB bank (outputs must never
    cross a bank boundary).
  - gate logits accumulate into [128, 1024] two-bank PSUM slabs so the
    sigmoid runs as 144 big activations instead of 288 small ones (the
    scalar engine pays a ~352-cycle pipeline bubble per instruction).
  - relu and the final gate*x multiply run on the DVE (2x bf16 mode),
    keeping the scalar engine free for the sigmoid.
  - deep rings (gate PSUM x3, DMA tiles x4) keep every engine
    streaming through throttle bursts; shallow rings here cost ~100us.
"""

import sys

for _p in ("/opt/trn_rl_repo",):
    if _p not in sys.path:
        sys.path.insert(0, _p)

import ml_dtypes
import numpy as np

import concourse.bacc as bacc
import concourse.bass as bass
import concourse.mybir as mybir
from concourse import tile
from concourse.bass_utils import run_bass_kernel_spmd

B, C, H, W = 8, 512, 192, 192
R = 32            # C // RED
NPIX = H * W      # 36864 pixels per batch element
N_CORES = 8
PART = 128
G = C // PART     # 4 channel groups
CH = 512          # pixels per compute chunk = one PSUM bank of fp32
DMA_N = 4 * CH    # pixels per DMA tile (2048)

F32 = mybir.dt.float32
BF16 = mybir.dt.bfloat16
AF = mybir.ActivationFunctionType
BF16_NP = ml_dtypes.bfloat16


def build(npix: int = NPIX, dma_n: int = DMA_N):
    """Build the per-core Bass program (SPMD: identical on all cores)."""
    assert npix % dma_n == 0
    # Small tiles at the head (prime the out-DMA stream sooner: the out
    # ring is the critical path and can only start after the first tile's
    # compute) and at the tail (let the final transfers drain sooner).
    half = dma_n // 2
    assert half % (2 * CH) == 0
    tile_sizes = [half] * 2 + [dma_n] * (npix // dma_n - 2) + [half] * 2
    assert sum(tile_sizes) == npix

    nc = bacc.Bacc("TRN2", target_bir_lowering=False, debug=False, num_devices=N_CORES)

    x_d = nc.dram_tensor("x", [C, npix], BF16, kind="ExternalInput").ap()
    w1t_d = nc.dram_tensor("w1t", [C, R], BF16, kind="ExternalInput").ap()  # W1.T
    w2r_d = nc.dram_tensor("w2r", [PART, C], BF16, kind="ExternalInput").ap()
    y_d = nc.dram_tensor("y", [C, npix], BF16, kind="ExternalOutput").ap()

    with tile.TileContext(nc) as tc:
        with (
            tc.tile_pool(name="wpool", bufs=1) as wpool,
            tc.tile_pool(name="xp", bufs=5) as xp,
            tc.tile_pool(name="hp", bufs=8) as hp,
            tc.tile_pool(name="gp", bufs=6) as gp,
            tc.tile_pool(name="op", bufs=4) as op_,
            tc.tile_pool(name="pp", bufs=2, space=bass.MemorySpace.PSUM) as pp,
        ):
            # Weights, loaded once.
            # w1t[p, g, r] = W1T[g*128+p, r]; w2r[32c+r, g, m] = W2T[r, g*128+m]
            # (only strip 0 of the replicated W2 is used here).
            w1t = wpool.tile([PART, G, R], BF16)
            nc.sync.dma_start(w1t[:], w1t_d.rearrange("(g p) r -> p g r", p=PART))
            w2r = wpool.tile([PART, G, PART], BF16)
            nc.sync.dma_start(w2r[:], w2r_d.rearrange("p (g m) -> p g m", m=PART))

            n0 = 0
            for ti, tn in enumerate(tile_sizes):
                nch = tn // CH
                xt = xp.tile([PART, G, tn], BF16, tag="xt")
                nc.sync.dma_start(
                    xt[:],
                    x_d[:, n0 : n0 + tn].rearrange("(g p) n -> p g n", p=PART),
                )

                # mm1 per 512-pixel chunk; relu on the DVE.
                hss = []
                for k in range(nch):
                    sl = slice(k * CH, (k + 1) * CH)
                    hps = pp.tile([R, CH], F32, tag="hps")
                    for g in range(G):
                        nc.tensor.matmul(
                            hps[:], w1t[:, g, :], xt[:, g, sl],
                            start=(g == 0), stop=(g == G - 1),
                        )
                    hs = hp.tile([R, CH], BF16, tag="hs")
                    nc.vector.tensor_scalar_max(hs[:], hps[:], 0.0)
                    hss.append(hs)

                ot = op_.tile([PART, G, tn], BF16, tag="ot")
                for g in range(G):
                    for hf in range(nch // 2):
                        # Two chunk matmuls (same stationary W2 block) fill a
                        # [128, 1024] gate-logit slab, one 2KB bank each.
                        gps = pp.tile([PART, 2 * CH], F32, tag="gps", bufs=3)
                        for i in range(2):
                            nc.tensor.matmul(
                                gps[:, i * CH : (i + 1) * CH],
                                w2r[0:R, g, :],
                                hss[2 * hf + i][:],
                                start=True, stop=True,
                            )
                        sl2 = slice(hf * 2 * CH, (hf + 1) * 2 * CH)
                        gs = gp.tile([PART, 2 * CH], BF16, tag="gs")
                        nc.scalar.activation(gs[:], gps[:], AF.Sigmoid)
                        nc.vector.tensor_mul(ot[:, g, sl2], gs[:], xt[:, g, sl2])

                # Drain assist: the sync ring's loads finish before the tail
                # stores, so route the last few stores onto it -- both HWDGE
                # rings then drain the output in parallel.
                out_eng = nc.sync if ti >= len(tile_sizes) - 3 else nc.scalar
                out_eng.dma_start(
                    y_d[:, n0 : n0 + tn].rearrange("(g p) n -> p g n", p=PART),
                    ot[:],
                )
                n0 += tn

    nc.compile()
    return nc


def _plausible(y: np.ndarray, x: np.ndarray) -> bool:
    """Cheap integrity check: y = sigmoid(.)*x implies |y| <= |x| (modulo
    bf16 rounding), finite everywhere, and y is never 0 where x isn't
    tiny (the gate can't underflow for this weight scale). Transient DMA
    corruption / stale pages violate these with near-certainty."""
    y = np.asarray(y, dtype=np.float32)
    x = np.asarray(x, dtype=np.float32)
    if not np.isfinite(y).all():
        return False
    ax = np.abs(x)
    if (np.abs(y) > ax * 1.01 + 1e-30).any():
        return False
    if np.count_nonzero((y == 0.0) & (ax > 1e-3)) > y.size // 1_000_000:
        return False
    return True


def kernel(x: np.ndarray, W1: np.ndarray, W2: np.ndarray, **run_kwargs):
    """Full-input entry point: shards batch over 8 cores, returns full output."""
    x = np.asarray(x)
    assert x.shape == (B, C, H, W), x.shape
    nc = build()

    w1t = np.ascontiguousarray(np.asarray(W1).T).astype(BF16_NP)  # [512, 32]
    w2r = np.ascontiguousarray(np.tile(np.asarray(W2).T, (4, 1))).astype(BF16_NP)
    x_bf = [
        np.ascontiguousarray(x[i].reshape(C, NPIX)).astype(BF16_NP)
        for i in range(N_CORES)
    ]
    in_maps = [{"x": x_bf[i], "w1t": w1t, "w2r": w2r} for i in range(N_CORES)]
    retries = 2 if not run_kwargs.get("trace") else 0
    for attempt in range(retries + 1):
        res = run_bass_kernel_spmd(nc, in_maps, list(range(N_CORES)), **run_kwargs)
        if all(
            _plausible(res.results[i]["y"], x_bf[i]) for i in range(N_CORES)
        ):
            break
    y = np.stack(
        [
            res.results[i]["y"].astype(np.float32).reshape(C, H, W)
            for i in range(N_CORES)
        ]
    )
    if run_kwargs:
        return y, res
    return y
